# revision 1
# baseline (speedup 1.0000x reference)
"""GCMCGraphConv Trainium2 kernel (8 NeuronCores, SPMD).

Sharding: destination-partitioned edge parallelism. Edges are sorted by
edge_dst on the host and split into 8 contiguous dst ranges (12500 dst
rows per core). The review-embedding table and feat table are replicated
to every core; each core gathers the rows for its own edges with
indirect DMA, runs the per-edge review MLP + gating on-chip, and
scatter-sums messages into its own dst rows with one-hot matmuls.
No collectives are needed; the host concatenates the 8 output shards.

Per-core layout: a core's edges are grouped by blocks of 256 consecutive
destination rows ("groups"). Each group's edge list is padded to a
multiple of 128 (the SPMD program is identical across cores, so the
per-group capacity is the max count across the 8 cores). Padding edges
carry w=0 and dst_rel=-1 so they contribute nothing.

Numerics: gathers and PSUM accumulation stay fp32 (feat rows are cast to
bf16 in-flight by the SWDGE gather); matmul operands and messages are
bf16. The MLP runs on pairs of 128-edge tiles (free dim 256) to halve
stationary loads; sigmoid is batched once per group so the scalar
engine's activation table stays on Gelu.
"""

import os

import numpy as np

P = 128          # partitions / edge-tile size
FEAT = 256
REV_DIM = 128
GROUP = 512      # dst rows per scatter group
N_CORES = 8

_prog_cache = {}


def _build_program(rev_vocab, n_src, caps):
    from concourse import bass, tile, mybir, bacc

    NG = len(caps)
    n_slots = int(sum(caps))
    maxt = max(int(c) // P for c in caps)
    out_rows = NG * GROUP
    f32 = mybir.dt.float32
    bf16 = mybir.dt.bfloat16
    i32 = mybir.dt.int32

    nc = bacc.Bacc(None, target_bir_lowering=False, debug=False)

    rev_emb = nc.declare_dram_parameter("rev_emb", [rev_vocab, REV_DIM], f32, isOutput=False)
    feat = nc.declare_dram_parameter("feat", [n_src, FEAT], f32, isOutput=False)
    emeta = nc.declare_dram_parameter("emeta", [n_slots, 4], i32, isOutput=False)
    rw1t = nc.declare_dram_parameter("rw1t", [REV_DIM, FEAT], bf16, isOutput=False)
    rw2t = nc.declare_dram_parameter("rw2t", [P, 2 * FEAT], bf16, isOutput=False)
    rw3t = nc.declare_dram_parameter("rw3t", [P, 2 * FEAT], bf16, isOutput=False)
    pwsw = nc.declare_dram_parameter("pwsw", [REV_DIM, 2], bf16, isOutput=False)
    lwt = nc.declare_dram_parameter("lwt", [P, 2 * FEAT], bf16, isOutput=False)
    linb = nc.declare_dram_parameter("linb", [1, FEAT], bf16, isOutput=False)
    ones1 = nc.declare_dram_parameter("ones1", [1, P], bf16, isOutput=False)
    ident = nc.declare_dram_parameter("ident", [P, P], f32, isOutput=False)
    iota = nc.declare_dram_parameter("iota", [P, GROUP], f32, isOutput=False)
    out = nc.declare_dram_parameter("out", [out_rows, FEAT], f32, isOutput=True)

    AF = mybir.ActivationFunctionType
    OP = mybir.AluOpType

    with tile.TileContext(nc) as tc:
        with tc.tile_pool(name="const", bufs=1) as cpool, \
             tc.tile_pool(name="sb", bufs=4) as sb, \
             tc.tile_pool(name="stage", bufs=3) as stg, \
             tc.tile_pool(name="grp", bufs=3) as grp, \
             tc.tile_pool(name="msgp", bufs=4) as msgp, \
             tc.tile_pool(name="rfep", bufs=8) as rfep, \
             tc.tile_pool(name="ps", bufs=2, space="PSUM") as ps, \
             tc.tile_pool(name="psbc", bufs=2, space="PSUM") as psbc, \
             tc.tile_pool(name="psh", bufs=1, space="PSUM") as psh:

            c_rw1t = cpool.tile([REV_DIM, FEAT], bf16)
            nc.sync.dma_start(out=c_rw1t[:], in_=rw1t[:])
            c_rw2t = cpool.tile([P, 2 * FEAT], bf16)
            nc.sync.dma_start(out=c_rw2t[:], in_=rw2t[:])
            c_rw3t = cpool.tile([P, 2 * FEAT], bf16)
            nc.sync.dma_start(out=c_rw3t[:], in_=rw3t[:])
            c_pwsw = cpool.tile([REV_DIM, 2], bf16)
            nc.sync.dma_start(out=c_pwsw[:], in_=pwsw[:])
            c_lwt = cpool.tile([P, 2 * FEAT], bf16)
            nc.sync.dma_start(out=c_lwt[:], in_=lwt[:])
            c_linb = cpool.tile([1, FEAT], bf16)
            nc.sync.dma_start(out=c_linb[:], in_=linb[:])
            c_ones = cpool.tile([1, P], bf16)
            nc.sync.dma_start(out=c_ones[:], in_=ones1[:])
            c_id = cpool.tile([P, P], f32)
            nc.sync.dma_start(out=c_id[:], in_=ident[:])
            c_iota = cpool.tile([P, GROUP], f32)
            nc.sync.dma_start(out=c_iota[:], in_=iota[:])

            slot = 0

            def pass1(g, slot):
                ntile = int(caps[g]) // P
                prs = grp.tile([P, 2 * maxt], f32, tag="prs")
                em_g = stg.tile([P, maxt, 4], i32, tag="em")
                nc.sync.dma_start(
                    out=em_g[:, 0:ntile, :],
                    in_=emeta[slot:slot + ntile * P, :].rearrange(
                        "(n p) d -> p n d", p=P))
                ems = [em_g[:, t, :] for t in range(ntile)]
                ftes, rfbs = [], []
                chunk_of, off_of = {}, {}
                t = 0
                pi = 0
                while t < ntile:
                    ct = 2 if t + 1 < ntile else 1
                    W = ct * P
                    rfes = []
                    for k in range(ct):
                        r0 = slot + (t + k) * P
                        rfe = rfep.tile([P, REV_DIM], f32, tag=f"rfe{k}")
                        nc.gpsimd.indirect_dma_start(
                            out=rfe[:], out_offset=None, in_=rev_emb[:],
                            in_offset=bass.IndirectOffsetOnAxis(
                                ap=ems[t + k][:, 0:1], axis=0))
                        rfes.append(rfe)
                        fte = stg.tile([P, FEAT], bf16, tag=f"fte{t + k}")
                        nc.gpsimd.indirect_dma_start(
                            out=fte[:], out_offset=None, in_=feat[:],
                            in_offset=bass.IndirectOffsetOnAxis(
                                ap=ems[t + k][:, 1:2], axis=0))
                        ftes.append(fte)
                    p_rT = ps.tile([P, 2 * P], f32, tag="pA")
                    for k in range(ct):
                        nc.tensor.transpose(out=p_rT[:, k * P:(k + 1) * P],
                                            in_=rfes[k][:], identity=c_id[:])
                    rT = sb.tile([P, 2 * P], bf16, tag="rT")
                    nc.vector.tensor_copy(out=rT[:, 0:W], in_=p_rT[:, 0:W])
                    p_pr = ps.tile([P, 2 * P], f32, tag="pA")
                    for k in range(ct):
                        nc.tensor.matmul(out=p_pr[:, 2 * k:2 * k + 2],
                                         lhsT=rT[:, k * P:(k + 1) * P], rhs=c_pwsw[:],
                                         start=True, stop=True)
                    nc.vector.tensor_copy(out=prs[:, 2 * t:2 * t + 2 * ct],
                                          in_=p_pr[:, 0:2 * ct])
                    a1 = []
                    for m in range(2):
                        pa1 = psbc.tile([P, 2 * P], f32, tag="pBC")
                        nc.tensor.matmul(out=pa1[:, 0:W],
                                         lhsT=c_rw1t[:, m * P:(m + 1) * P],
                                         rhs=rT[:, 0:W], start=True, stop=True)
                        a1m = sb.tile([P, 2 * P], bf16, tag=f"a1_{m}")
                        nc.scalar.activation(out=a1m[:, 0:W], in_=pa1[:, 0:W], func=AF.Gelu)
                        a1.append(a1m)
                    a2 = []
                    for m in range(2):
                        pa2 = psbc.tile([P, 2 * P], f32, tag="pBC")
                        for j in range(2):
                            nc.tensor.matmul(
                                out=pa2[:, 0:W],
                                lhsT=c_rw2t[:, j * FEAT + m * P: j * FEAT + (m + 1) * P],
                                rhs=a1[j][:, 0:W], start=(j == 0), stop=(j == 1))
                        a2m = sb.tile([P, 2 * P], bf16, tag=f"a2_{m}")
                        nc.scalar.activation(out=a2m[:, 0:W], in_=pa2[:, 0:W], func=AF.Gelu)
                        a2.append(a2m)
                    p_rf = ps.tile([P, 2 * FEAT], f32, tag="pDE")
                    for k in range(ct):
                        for j in range(2):
                            nc.tensor.matmul(
                                out=p_rf[:, k * FEAT:(k + 1) * FEAT],
                                lhsT=a2[j][:, k * P:(k + 1) * P],
                                rhs=c_rw3t[:, j * FEAT:(j + 1) * FEAT],
                                start=(j == 0), stop=(j == 1))
                    rfb = stg.tile([P, 2 * FEAT], bf16, tag=f"rfb{pi}")
                    nc.scalar.activation(out=rfb[:, 0:ct * FEAT], in_=p_rf[:, 0:ct * FEAT],
                                         func=AF.Copy)
                    rfbs.append(rfb)
                    for k in range(ct):
                        chunk_of[t + k] = pi
                        off_of[t + k] = k
                    t += ct
                    pi += 1
                sgm = grp.tile([P, 2 * maxt], f32, tag="sgm")
                nc.scalar.activation(out=sgm[:, 0:2 * ntile], in_=prs[:, 0:2 * ntile],
                                     func=AF.Sigmoid)
                return dict(g=g, ntile=ntile, ems=ems, ftes=ftes, rfbs=rfbs,
                            sgm=sgm, chunk_of=chunk_of, off_of=off_of)

            def pass2(st):
                g = st["g"]
                ntile = st["ntile"]
                ems, ftes, rfbs = st["ems"], st["ftes"], st["rfbs"]
                sgm, chunk_of, off_of = st["sgm"], st["chunk_of"], st["off_of"]
                h0 = psh.tile([P, GROUP], f32, tag="h0")
                h1 = psh.tile([P, GROUP], f32, tag="h1")
                for t in range(ntile):
                    em = ems[t]
                    ab = msgp.tile([P, 2], f32, tag="ab")
                    nc.vector.tensor_scalar(out=ab[:], in0=sgm[:, 2 * t:2 * t + 2],
                                            scalar1=em[:, 2:3].bitcast(f32),
                                            scalar2=None, op0=OP.mult)
                    t1 = msgp.tile([P, FEAT], bf16, tag="t1")
                    nc.vector.tensor_scalar(out=t1[:], in0=ftes[t][:], scalar1=ab[:, 0:1],
                                            scalar2=None, op0=OP.mult)
                    t2 = msgp.tile([P, FEAT], bf16, tag="t2")
                    nc.vector.tensor_scalar(
                        out=t2[:],
                        in0=rfbs[chunk_of[t]][:, off_of[t] * FEAT:(off_of[t] + 1) * FEAT],
                        scalar1=ab[:, 1:2], scalar2=None, op0=OP.mult)
                    S = msgp.tile([P, GROUP], bf16, tag="S")
                    nc.vector.tensor_scalar(out=S[:], in0=c_iota[:],
                                            scalar1=em[:, 3:4].bitcast(f32),
                                            scalar2=None, op0=OP.is_equal)
                    nc.tensor.matmul(out=h0[:], lhsT=t1[:, 0:P], rhs=S[:],
                                     start=(t == 0), stop=False)
                    nc.tensor.matmul(out=h0[:], lhsT=t2[:, 0:P], rhs=S[:],
                                     start=False, stop=(t == ntile - 1))
                    nc.tensor.matmul(out=h1[:], lhsT=t1[:, P:FEAT], rhs=S[:],
                                     start=(t == 0), stop=False)
                    nc.tensor.matmul(out=h1[:], lhsT=t2[:, P:FEAT], rhs=S[:],
                                     start=False, stop=(t == ntile - 1))
                hs = sb.tile([P, 2 * GROUP], bf16, tag="hs")
                nc.vector.tensor_copy(out=hs[:, 0:GROUP], in_=h0[:])
                nc.vector.tensor_copy(out=hs[:, GROUP:2 * GROUP], in_=h1[:])
                for b in range(GROUP // P):
                    po = ps.tile([P, 2 * FEAT], f32, tag="pDE")
                    nc.tensor.matmul(out=po[:, 0:FEAT], lhsT=hs[:, b * P:(b + 1) * P],
                                     rhs=c_lwt[:, 0:FEAT], start=True, stop=False)
                    nc.tensor.matmul(out=po[:, 0:FEAT], lhsT=hs[:, GROUP + b * P:GROUP + (b + 1) * P],
                                     rhs=c_lwt[:, FEAT:2 * FEAT], start=False, stop=False)
                    nc.tensor.matmul(out=po[:, 0:FEAT], lhsT=c_ones[0:1, :], rhs=c_linb[0:1, :],
                                     start=False, stop=True)
                    ot = sb.tile([P, FEAT], f32, tag="ot")
                    nc.vector.tensor_copy(out=ot[:], in_=po[:, 0:FEAT])
                    nc.sync.dma_start(
                        out=out[g * GROUP + b * P: g * GROUP + (b + 1) * P, :],
                        in_=ot[:])

            prev = None
            for g in range(NG):
                st = pass1(g, slot)
                slot += int(caps[g])
                if prev is not None:
                    pass2(prev)
                prev = st
            pass2(prev)
    nc.compile()
    return nc


def kernel(**inputs):
    import ml_dtypes
    from concourse.bass_utils import run_bass_kernel_spmd

    feat = np.asarray(inputs["feat"], dtype=np.float32)
    cj = np.asarray(inputs["cj"], dtype=np.float32)
    ci = np.asarray(inputs["ci"], dtype=np.float32)
    edge_src = np.asarray(inputs["edge_src"]).astype(np.int64)
    edge_dst = np.asarray(inputs["edge_dst"]).astype(np.int64)
    review_id = np.asarray(inputs["review_id"]).astype(np.int64)
    rev_emb = np.asarray(inputs["review_emb"], dtype=np.float32)
    prob_w = np.asarray(inputs["prob_w"], dtype=np.float32)
    score_w = np.asarray(inputs["score_w"], dtype=np.float32)
    rw1 = np.asarray(inputs["rw1"], dtype=np.float32)
    rw2 = np.asarray(inputs["rw2"], dtype=np.float32)
    rw3 = np.asarray(inputs["rw3"], dtype=np.float32)
    lin_w = np.asarray(inputs["lin_w"], dtype=np.float32)
    lin_b = np.asarray(inputs["lin_b"], dtype=np.float32)

    n_src = feat.shape[0]
    n_dst = ci.shape[0]
    rev_vocab = rev_emb.shape[0]
    dst_per_core = n_dst // N_CORES
    ng = -(-dst_per_core // GROUP)
    bf = ml_dtypes.bfloat16

    order = np.argsort(edge_dst, kind="stable")
    s_src = edge_src[order]
    s_dst = edge_dst[order]
    s_rev = review_id[order]
    s_w = (cj[s_src, 0] * ci[s_dst, 0]).astype(np.float32)

    core_of = s_dst // dst_per_core
    core_start = np.searchsorted(core_of, np.arange(N_CORES), side="left")
    core_end = np.searchsorted(core_of, np.arange(N_CORES), side="right")

    counts = np.zeros((N_CORES, ng), dtype=np.int64)
    group_starts = np.zeros((N_CORES, ng), dtype=np.int64)
    for c in range(N_CORES):
        lo, hi = core_start[c], core_end[c]
        dloc = s_dst[lo:hi] - c * dst_per_core
        gid = dloc // GROUP
        counts[c] = np.bincount(gid, minlength=ng)
        group_starts[c] = lo + np.concatenate(([0], np.cumsum(counts[c])[:-1]))

    caps = (np.maximum(1, -(-counts.max(axis=0) // P)) * P).astype(np.int64)
    n_slots = int(caps.sum())

    consts = dict(
        rw1t=np.ascontiguousarray(rw1.T).astype(bf),
        rw2t=np.ascontiguousarray(
            np.concatenate([rw2.T[0:P, :], rw2.T[P:2 * P, :]], axis=1)).astype(bf),
        rw3t=np.ascontiguousarray(
            np.concatenate([rw3.T[0:P, :], rw3.T[P:2 * P, :]], axis=1)).astype(bf),
        pwsw=np.ascontiguousarray(
            np.concatenate([prob_w, score_w], axis=0).T).astype(bf),
        lwt=np.ascontiguousarray(
            np.concatenate([lin_w.T[0:P, :], lin_w.T[P:2 * P, :]], axis=1)).astype(bf),
        linb=lin_b.reshape(1, FEAT).astype(bf),
        ones1=np.ones((1, P), dtype=bf),
        ident=np.eye(P, dtype=np.float32),
        iota=np.broadcast_to(np.arange(GROUP, dtype=np.float32), (P, GROUP)).copy(),
    )

    in_maps = []
    for c in range(N_CORES):
        emeta = np.zeros((n_slots, 4), dtype=np.int32)
        emeta[:, 3] = np.float32(-1.0).view(np.int32)
        slot = 0
        for g in range(ng):
            n = int(counts[c, g])
            lo = int(group_starts[c, g])
            emeta[slot:slot + n, 0] = s_rev[lo:lo + n]
            emeta[slot:slot + n, 1] = s_src[lo:lo + n]
            emeta[slot:slot + n, 2] = s_w[lo:lo + n].view(np.int32)
            emeta[slot:slot + n, 3] = (
                (s_dst[lo:lo + n] - c * dst_per_core - g * GROUP)
                .astype(np.float32).view(np.int32))
            slot += int(caps[g])
        im = dict(rev_emb=rev_emb, feat=feat, emeta=emeta, **consts)
        in_maps.append(im)

    key = (rev_vocab, n_src, tuple(int(x) for x in caps))
    if key not in _prog_cache:
        _prog_cache[key] = _build_program(rev_vocab, n_src, caps)
    nc = _prog_cache[key]

    trace = bool(os.environ.get("BASS_KERNEL_TRACE"))
    res = run_bass_kernel_spmd(nc, in_maps, core_ids=list(range(N_CORES)),
                               trace=trace)
    global last_results
    last_results = res
    out = np.concatenate(
        [res.results[c]["out"][:dst_per_core] for c in range(N_CORES)], axis=0)
    return out.astype(np.float32)


last_results = None

